# revision 2
# baseline (speedup 1.0000x reference)
"""Trainium2 Bass kernel for nn_BlocksCore (RIMs BlocksCore fwd step).

Contract: kernel(**inputs) takes FULL unsharded inputs (np arrays, keyed as in
setup_inputs) and returns the FULL output tuple (hx_out [8192,1024] f32,
mask_full [8192,1024] f32), matching reference().

Strategy: pure data-parallel over batch (1024 samples/core on 8 cores).
Device layout is feature-major ([features, batch]); the host pre-transposes
inputs / post-transposes outputs and pre-fuses weights (Wv1[1] @ gru_wi).
"""

import numpy as np
import ml_dtypes
from contextlib import ExitStack

import concourse.bass as bass
import concourse.bacc as bacc
import concourse.tile as tile
import concourse.mybir as mybir
from concourse.bass_utils import run_bass_kernel_spmd

AF = mybir.ActivationFunctionType
OP = mybir.AluOpType
f32 = mybir.dt.float32
bf16 = mybir.dt.bfloat16
BF = ml_dtypes.bfloat16

B, NINP, NHID = 8192, 256, 1024
NCORES = 8
BC = B // NCORES          # 1024 per core
F = 512                   # batch-tile columns
NT = BC // F              # 2 tiles
NB = 8                    # output blocks
BS = 128                  # block size
NH, DH = 4, 16            # comm-attn heads


def _build_consts():
    """Constant 0/1 selector matrices (bf16)."""
    c = {}
    # s1 partition-sum: prod[p] [128=(a2,e64), F] -> s1 [8, F]; col 2p+a
    m = np.zeros((4, 128, 8), BF)
    for p in range(4):
        m[p, 0:64, 2 * p] = 1
        m[p, 64:128, 2 * p + 1] = 1
    c["c_s1sum"] = m.transpose(1, 0, 2).reshape(128, 32)  # slice [:, p*8:(p+1)*8]

    # replication [8 -> 128]: slice k gives row k -> all 128 rows
    m = np.zeros((8, 8, 128), BF)
    for k in range(8):
        m[k, k, :] = 1
    c["reps"] = m.transpose(1, 0, 2).reshape(8, 8 * 128)  # [:, k*128:(k+1)*128]

    # mask diff: diff[8i+j] = s1[j] - s1[i]
    pq = np.zeros((8, 64), BF)
    for i in range(8):
        for j in range(8):
            pq[j, 8 * i + j] += 1
            pq[i, 8 * i + j] -= 1
    c["pq"] = pq

    # rank: rank[i] = sum_j g[8i+j]
    r64 = np.zeros((64, 8), BF)
    for i in range(8):
        for j in range(8):
            r64[8 * i + j, i] = 1
    c["r64"] = r64

    # comm-attn QK sum: prod2(i,r) rows (a,h,d)=64a+16h+d -> s_i row 8j+h, j=2r+a
    m = np.zeros((4, 128, 32), BF)
    for r in range(4):
        for a in range(2):
            for h in range(4):
                for d in range(16):
                    m[r, 64 * a + 16 * h + d, 4 * (2 * r + a) + h] = 1
    c["c_qksum"] = m.transpose(1, 0, 2).reshape(128, 128)  # [:, r*32:(r+1)*32]

    # denom: expS_i row 4j+h -> denom row 4i+h (sum over j)
    m = np.zeros((8, 32, 32), BF)
    for i in range(8):
        for j in range(8):
            for h in range(4):
                m[i, 4 * j + h, 4 * i + h] = 1
    c["c_den"] = m.transpose(1, 0, 2).reshape(32, 8 * 32)  # [:, i*32:(i+1)*32]

    # exp replication: expS_i row 8(2r+a)+h -> rep row 64a+16h+d
    m = np.zeros((4, 32, 128), BF)
    for r in range(4):
        for a in range(2):
            for h in range(4):
                for d in range(16):
                    m[r, 4 * (2 * r + a) + h, 64 * a + 16 * h + d] = 1
    c["c_erep"] = m.transpose(1, 0, 2).reshape(32, 4 * 128)  # [:, r*128:(r+1)*128]

    # fold: avp rows 64a+16h+d -> o rows 16h+d (sum over a)
    fold = np.zeros((128, 64), BF)
    for a in range(2):
        for h in range(4):
            for d in range(16):
                fold[64 * a + 16 * h + d, 16 * h + d] = 1
    c["fold"] = fold

    # recip replication: recipS row 4(2c+a)+h -> rep row 64a+16h+d
    m = np.zeros((4, 32, 128), BF)
    for cc in range(4):
        for a in range(2):
            for h in range(4):
                for d in range(16):
                    m[cc, 4 * (2 * cc + a) + h, 64 * a + 16 * h + d] = 1
    c["c_rrep"] = m.transpose(1, 0, 2).reshape(32, 4 * 128)  # [:, c*128:(c+1)*128]
    return c


_CONSTS = _build_consts()
_PROGRAM = None  # (nc, ...) cache


def _build_program():
    nc = bacc.Bacc("TRN2", target_bir_lowering=False, debug=False)

    def din(name, shape, dt=bf16):
        return nc.dram_tensor(name, shape, dt, kind="ExternalInput")

    # per-core activations
    inpT = din("inpT", [NINP, BC])            # bf16
    inpTf = din("inpTf", [NINP, BC], f32)
    hxT = din("hxT", [NHID, BC], f32)
    hxTb = din("hxTb", [NHID, BC])            # bf16
    # weights (shared)
    wq1 = din("wq1", [128, 512], f32); wk1 = din("wk1", [128, 128], f32)
    csf = {}
    for nmm in ("c_s1sum", "pq", "r64"):
        csf[nmm] = nc.dram_tensor("f_" + nmm, list(_CONSTS[nmm].shape), f32,
                                  kind="ExternalInput")
    wfu = din("wfu", [128, 6144]); wh = din("wh", [128, 3072])
    wq2 = din("wq2", [128, 512]); wk2 = din("wk2", [128, 512]); wv2 = din("wv2", [128, 512])
    fcg = din("fcg", [64, 256])
    # biases f32 [128, n]
    b_rz = din("b_rz", [128, 16], f32)        # cols 2k: r, 2k+1: -z (negated)
    b_nbh = din("b_nbh", [128, 8], f32)
    b_nbi = din("b_nbi", [128, 8], f32)
    b_fg = din("b_fg", [128, 2], f32)
    # consts
    cs = {k: din("c_" + k, list(v.shape)) for k, v in _CONSTS.items()}

    houtT = nc.dram_tensor("houtT", [NHID, BC], f32, kind="ExternalOutput")
    mask8 = nc.dram_tensor("mask8", [8, BC], f32, kind="ExternalOutput")
    import os
    DEBUG = bool(os.environ.get("KDEBUG"))
    dbg = {}
    if DEBUG:
        dbg["s1"] = nc.dram_tensor("d_s1", [8, BC], f32, kind="ExternalOutput")
        dbg["att0"] = nc.dram_tensor("d_att0", [128, BC], f32, kind="ExternalOutput")
        dbg["hpr"] = nc.dram_tensor("d_hpr", [NHID, BC], f32, kind="ExternalOutput")
        dbg["exps"] = nc.dram_tensor("d_exps", [256, BC], f32, kind="ExternalOutput")
        dbg["o"] = nc.dram_tensor("d_o", [512, BC], f32, kind="ExternalOutput")
        dbg["ze0"] = nc.dram_tensor("d_ze0", [128, BC], f32, kind="ExternalOutput")

    with ExitStack() as ctx:
        tc = ctx.enter_context(tile.TileContext(nc))
        wp = ctx.enter_context(tc.tile_pool(name="wp", bufs=1))       # weights
        sb = ctx.enter_context(tc.tile_pool(name="sb", bufs=1))       # per-tile inputs
        ak = ctx.enter_context(tc.tile_pool(name="ak", bufs=4))       # per-k transients
        ps = ctx.enter_context(tc.tile_pool(name="ps", bufs=5, space="PSUM"))
        ps2 = ctx.enter_context(tc.tile_pool(name="ps2", bufs=3, space="PSUM"))

        def wtile(dram, shape, dt=bf16):
            t = wp.tile(shape, dt, tag=dram.name, name="t")
            nc.sync.dma_start(t[:], dram.ap())
            return t

        W = {}
        CF = {k: wtile(v, list(_CONSTS[k].shape), f32) for k, v in csf.items()}
        for d, sh in [(wq1, [128, 512]), (wk1, [128, 128]), (wfu, [128, 6144]),
                      (wh, [128, 3072]), (wq2, [128, 512]), (wk2, [128, 512]),
                      (wv2, [128, 512])]:
            dt_w = f32 if d.name in ("wq1", "wk1") else bf16
            W[d.name] = wtile(d, sh, dt_w)
        fcg_t = wp.tile([128, 256], bf16, tag="fcg", name="fcg")
        nc.sync.dma_start(fcg_t[0:64, :], fcg.ap())
        nc.sync.dma_start(fcg_t[64:128, :], fcg.ap())
        W["fcg"] = fcg_t
        for d, sh in [(b_rz, [128, 16]), (b_nbh, [128, 8]), (b_nbi, [128, 8]),
                      (b_fg, [128, 2])]:
            W[d.name] = wtile(d, sh, f32)
        C = {k: wtile(cs[k], list(_CONSTS[k].shape)) for k in cs}

        inp_t = [None, None]
        hx_t = [None] * 8
        hxb_t = [None] * 8

        for t in range(NT):
            sl = bass.ts(t, F)
            # ---- loads ----
            inpf_t = [None, None]
            for cch in range(2):
                inp_t[cch] = sb.tile([128, F], bf16, tag=f"inp{cch}", name=f"inp{cch}")
                nc.sync.dma_start(inp_t[cch][:], inpT.ap()[bass.ts(cch, 128), sl])
                inpf_t[cch] = sb.tile([128, F], f32, tag=f"inpf{cch}", name=f"inpf{cch}")
                nc.sync.dma_start(inpf_t[cch][:], inpTf.ap()[bass.ts(cch, 128), sl])
            for k in range(8):
                hx_t[k] = sb.tile([128, F], f32, tag=f"hx{k}", name=f"hx{k}")
                nc.sync.dma_start(hx_t[k][:], hxT.ap()[bass.ts(k, 128), sl])
                hxb_t[k] = sb.tile([128, F], bf16, tag=f"hxb{k}", name=f"hxb{k}")
                nc.sync.dma_start(hxb_t[k][:], hxTb.ap()[bass.ts(k, 128), sl])

            # ---- phase A: input attention scores + mask ----
            # kkRep [128,F]: rows 0:64 and 64:128 both = inp @ Wk1[1]
            kk_ps = ps.tile([128, F], f32, tag="ps128", name="ps128")
            for cch in range(2):
                nc.tensor.matmul(kk_ps[0:64, :], W["wk1"][:, bass.ts(cch, 64)],
                                 inpf_t[cch][:], start=(cch == 0), stop=(cch == 1))
            for cch in range(2):
                nc.tensor.matmul(kk_ps[64:128, :], W["wk1"][:, bass.ts(cch, 64)],
                                 inpf_t[cch][:], start=(cch == 0), stop=(cch == 1),
                                 tile_position=(0, 64))
            kkS = sb.tile([128, F], f32, tag="kkS", name="kkS")
            nc.scalar.copy(kkS[:], kk_ps[:])

            prods = []
            for p in range(4):
                q_ps = ps.tile([128, F], f32, tag="ps128", name="ps128")
                nc.tensor.matmul(q_ps[0:64, :], W["wq1"][:, bass.ts(2 * p, 64)],
                                 hx_t[2 * p][:], start=True, stop=True)
                nc.tensor.matmul(q_ps[64:128, :], W["wq1"][:, bass.ts(2 * p + 1, 64)],
                                 hx_t[2 * p + 1][:], start=True, stop=True,
                                 tile_position=(0, 64))
                pr = ak.tile([128, F], f32, tag="prod", name="prod")
                nc.vector.tensor_tensor(pr[:], q_ps[:], kkS[:], OP.mult)
                prods.append(pr)

            s1_ps = ps2.tile([8, F], f32, tag="psS", name="psS")
            for p in range(4):
                nc.tensor.matmul(s1_ps[:], CF["c_s1sum"][:, bass.ts(p, 8)], prods[p][:],
                                 start=(p == 0), stop=(p == 3))
            s1S = sb.tile([8, F], f32, tag="s1S", name="s1S")
            nc.scalar.copy(s1S[:], s1_ps[:])
            s1Sb = sb.tile([8, F], bf16, tag="s1Sb", name="s1Sb")
            nc.scalar.copy(s1Sb[:], s1_ps[:])
            if DEBUG:
                nc.gpsimd.dma_start(dbg["s1"].ap()[:, sl], s1S[:])

            # mask: diff[8i+j] = s1[j]-s1[i]; g = diff>0; rank; mask = rank<=3
            diff_ps = ps2.tile([64, F], f32, tag="psS", name="psS")
            nc.tensor.matmul(diff_ps[:], CF["pq"][:], s1S[:], start=True, stop=True)
            g = sb.tile([64, F], f32, tag="g", name="g")
            nc.vector.tensor_single_scalar(g[:], diff_ps[:], 0.0, OP.is_gt)
            rank_ps = ps2.tile([8, F], f32, tag="psS", name="psS")
            nc.tensor.matmul(rank_ps[:], CF["r64"][:], g[:], start=True, stop=True)
            m8 = sb.tile([8, F], bf16, tag="m8", name="m8")
            nc.vector.tensor_single_scalar(m8[:], rank_ps[:], 3.5, OP.is_le)
            nc.gpsimd.dma_start(mask8.ap()[:, sl], m8[:])
            mrepS = [None] * 8
            for k in range(8):
                mr_ps = ps.tile([128, F], f32, tag="ps128", name="ps128")
                nc.tensor.matmul(mr_ps[:], C["reps"][:, bass.ts(k, 128)], m8[:],
                                 start=True, stop=True)
                mrepS[k] = sb.tile([128, F], bf16, tag=f"mrepS{k}", name=f"mrepS{k}")
                nc.scalar.copy(mrepS[k][:], mr_ps[:])

            # att_w replicated per block + input scaling
            attS = [None] * 8
            for k in range(8):
                a_ps = ps.tile([128, F], f32, tag="ps128", name="ps128")
                nc.tensor.matmul(a_ps[:], C["reps"][:, bass.ts(k, 128)], s1Sb[:],
                                 start=True, stop=True)
                attS[k] = sb.tile([128, F], bf16, tag=f"attS{k}", name=f"attS{k}")
                nc.scalar.activation(attS[k][:], a_ps[:], AF.Sigmoid, scale=0.125)
                if DEBUG and k == 0:
                    nc.gpsimd.dma_start(dbg["att0"].ap()[:, sl], attS[k][:])

            # ---- phase B: block GRU ----
            hpr = [None] * 8   # h' bf16
            zes = [None] * 8   # z'*(n-h) bf16
            for k in range(8):
                xk = [None, None]
                for cch in range(2):
                    xk[cch] = ak.tile([128, F], bf16, tag=f"xk{cch}", name=f"xk{cch}")
                    nc.vector.tensor_tensor(xk[cch][:], attS[k][:], inp_t[cch][:], OP.mult)
                kb = k * 384
                gate_ps = {}
                for gi, gn in enumerate(("r", "z", "n")):
                    gp = ps.tile([128, F], f32, tag="ps128", name="ps128")
                    last_wfu = gn == "n"
                    for cch in range(2):
                        nc.tensor.matmul(gp[:], W["wfu"][:, cch * 3072 + kb + gi * 128:
                                                         cch * 3072 + kb + gi * 128 + 128],
                                         xk[cch][:], start=(cch == 0),
                                         stop=(last_wfu and cch == 1))
                    if not last_wfu:
                        nc.tensor.matmul(gp[:], W["wh"][:, kb + gi * 128: kb + gi * 128 + 128],
                                         hxb_t[k][:], start=False, stop=True)
                    gate_ps[gn] = gp
                hn_ps = ps.tile([128, F], f32, tag="ps128", name="ps128")
                nc.tensor.matmul(hn_ps[:], W["wh"][:, kb + 256: kb + 384],
                                 hxb_t[k][:], start=True, stop=True)

                r = ak.tile([128, F], bf16, tag="r", name="r")
                nc.scalar.activation(r[:], gate_ps["r"][:], AF.Sigmoid,
                                     bias=W["b_rz"][:, 2 * k: 2 * k + 1])
                zp = ak.tile([128, F], bf16, tag="zp", name="zp")
                nc.scalar.activation(zp[:], gate_ps["z"][:], AF.Sigmoid, scale=-1.0,
                                     bias=W["b_rz"][:, 2 * k + 1: 2 * k + 2])
                rhn = ak.tile([128, F], bf16, tag="rhn", name="rhn")
                nc.vector.scalar_tensor_tensor(rhn[:], hn_ps[:],
                                               W["b_nbh"][:, k: k + 1], r[:],
                                               OP.add, OP.mult)
                npre = ak.tile([128, F], bf16, tag="npre", name="npre")
                nc.vector.tensor_tensor(npre[:], rhn[:], gate_ps["n"][:], OP.add)
                n = ak.tile([128, F], bf16, tag="n", name="n")
                nc.scalar.activation(n[:], npre[:], AF.Tanh,
                                     bias=W["b_nbi"][:, k: k + 1])
                e = ak.tile([128, F], bf16, tag="e", name="e")
                nc.vector.tensor_tensor(e[:], n[:], hxb_t[k][:], OP.subtract)
                zes[k] = sb.tile([128, F], bf16, tag=f"ze{k}", name=f"ze{k}")
                nc.vector.tensor_tensor(zes[k][:], zp[:], e[:], OP.mult)
                hpr[k] = sb.tile([128, F], bf16, tag=f"hpr{k}", name=f"hpr{k}")
                nc.vector.tensor_tensor(hpr[k][:], hxb_t[k][:], zes[k][:], OP.add)
                if DEBUG:
                    nc.gpsimd.dma_start(dbg["hpr"].ap()[bass.ts(k, 128), sl], hpr[k][:])
                    if k == 0:
                        nc.gpsimd.dma_start(dbg["ze0"].ap()[:, sl], zes[k][:])

            # ---- phase C: communication attention ----
            k2S = [None] * 4
            v2S = [None] * 4
            for rr in range(4):
                kp = ps.tile([128, F], f32, tag="ps128", name="ps128")
                nc.tensor.matmul(kp[0:64, :], W["wk2"][:, bass.ts(2 * rr, 64)],
                                 hpr[2 * rr][:], start=True, stop=True)
                nc.tensor.matmul(kp[64:128, :], W["wk2"][:, bass.ts(2 * rr + 1, 64)],
                                 hpr[2 * rr + 1][:], start=True, stop=True,
                                 tile_position=(0, 64))
                k2S[rr] = sb.tile([128, F], bf16, tag=f"k2S{rr}", name=f"k2S{rr}")
                nc.scalar.copy(k2S[rr][:], kp[:])
                vp = ps.tile([128, F], f32, tag="ps128", name="ps128")
                nc.tensor.matmul(vp[0:64, :], W["wv2"][:, bass.ts(2 * rr, 64)],
                                 hpr[2 * rr][:], start=True, stop=True)
                nc.tensor.matmul(vp[64:128, :], W["wv2"][:, bass.ts(2 * rr + 1, 64)],
                                 hpr[2 * rr + 1][:], start=True, stop=True,
                                 tile_position=(0, 64))
                v2S[rr] = sb.tile([128, F], bf16, tag=f"v2S{rr}", name=f"v2S{rr}")
                nc.scalar.copy(v2S[rr][:], vp[:])

            expS = [None] * 8
            for i in range(8):
                qp = ps.tile([128, F], f32, tag="ps128", name="ps128")
                nc.tensor.matmul(qp[0:64, :], W["wq2"][:, bass.ts(i, 64)],
                                 hpr[i][:], start=True, stop=True)
                nc.tensor.matmul(qp[64:128, :], W["wq2"][:, bass.ts(i, 64)],
                                 hpr[i][:], start=True, stop=True,
                                 tile_position=(0, 64))
                qdS = ak.tile([128, F], bf16, tag="qdS", name="qdS")
                nc.scalar.copy(qdS[:], qp[:])
                s_ps = ps2.tile([32, F], f32, tag="psS", name="psS")
                for rr in range(4):
                    pr2 = ak.tile([128, F], bf16, tag="prod2", name="prod2")
                    nc.vector.tensor_tensor(pr2[:], qdS[:], k2S[rr][:], OP.mult)
                    nc.tensor.matmul(s_ps[:], C["c_qksum"][:, bass.ts(rr, 32)], pr2[:],
                                     start=(rr == 0), stop=(rr == 3))
                expS[i] = sb.tile([32, F], bf16, tag=f"expS{i}", name=f"expS{i}")
                nc.scalar.activation(expS[i][:], s_ps[:], AF.Exp, scale=0.25)
                if DEBUG:
                    nc.gpsimd.dma_start(dbg["exps"].ap()[bass.ts(i, 32), sl], expS[i][:])

            den_ps = ps2.tile([32, F], f32, tag="psS", name="psS")
            for i in range(8):
                nc.tensor.matmul(den_ps[:], C["c_den"][:, bass.ts(i, 32)], expS[i][:],
                                 start=(i == 0), stop=(i == 7))
            recipF = sb.tile([32, F], f32, tag="recipF", name="recipF")
            with nc.allow_low_precision(reason="softmax denom ~8, approx recip ok"):
                nc.vector.reciprocal_approx_fast(recipF[:], den_ps[:])
            recipS = sb.tile([32, F], bf16, tag="recipS", name="recipS")
            nc.scalar.copy(recipS[:], recipF[:])

            oS = [None] * 4
            for cc in range(4):
                on_ps = ps.tile([128, F], f32, tag="ps128", name="ps128")
                for a in range(2):
                    i = 2 * cc + a
                    for rr in range(4):
                        erep_ps = ps.tile([128, F], f32, tag="ps128", name="ps128")
                        nc.tensor.matmul(erep_ps[:], C["c_erep"][:, bass.ts(rr, 128)],
                                         expS[i][:], start=True, stop=True)
                        avp = ak.tile([128, F], bf16, tag="avp", name="avp")
                        nc.vector.tensor_tensor(avp[:], erep_ps[:], v2S[rr][:], OP.mult)
                        if a == 0:
                            nc.tensor.matmul(on_ps[0:64, :], C["fold"][:], avp[:],
                                             start=(rr == 0), stop=(rr == 3))
                        else:
                            nc.tensor.matmul(on_ps[64:128, :], C["fold"][:], avp[:],
                                             start=(rr == 0), stop=(rr == 3),
                                             tile_position=(0, 64))
                rrep_ps = ps.tile([128, F], f32, tag="ps128", name="ps128")
                nc.tensor.matmul(rrep_ps[:], C["c_rrep"][:, bass.ts(cc, 128)],
                                 recipS[:], start=True, stop=True)
                rrepS = ak.tile([128, F], bf16, tag="rrepS", name="rrepS")
                nc.scalar.copy(rrepS[:], rrep_ps[:])
                oS[cc] = sb.tile([128, F], bf16, tag=f"oS{cc}", name=f"oS{cc}")
                nc.vector.tensor_tensor(oS[cc][:], on_ps[:], rrepS[:], OP.mult)
                if DEBUG:
                    nc.gpsimd.dma_start(dbg["o"].ap()[bass.ts(cc, 128), sl], oS[cc][:])

            # fc / gate + output combine
            for k in range(8):
                cc, a = divmod(k, 2)
                osrc = oS[cc][0:64, :] if a == 0 else oS[cc][64:128, :]
                wsl = W["fcg"][0:64, :] if a == 0 else W["fcg"][64:128, :]
                fc_ps = ps.tile([128, F], f32, tag="ps128", name="ps128")
                nc.tensor.matmul(fc_ps[:], wsl[:, 0:128], osrc, start=True, stop=True)
                gt_ps = ps.tile([128, F], f32, tag="ps128", name="ps128")
                nc.tensor.matmul(gt_ps[:], wsl[:, 128:256], osrc, start=True, stop=True)
                th = ak.tile([128, F], bf16, tag="th", name="th")
                nc.scalar.activation(th[:], fc_ps[:], AF.Tanh, bias=W["b_fg"][:, 0:1])
                sg = ak.tile([128, F], bf16, tag="sg", name="sg")
                nc.scalar.activation(sg[:], gt_ps[:], AF.Sigmoid, bias=W["b_fg"][:, 1:2])
                att = ak.tile([128, F], bf16, tag="att", name="att")
                nc.vector.tensor_tensor(att[:], sg[:], th[:], OP.mult)

                delta = ak.tile([128, F], bf16, tag="delta", name="delta")
                nc.vector.tensor_tensor(delta[:], zes[k][:], att[:], OP.add)
                mdelta = ak.tile([128, F], bf16, tag="mdelta", name="mdelta")
                nc.vector.tensor_tensor(mdelta[:], mrepS[k][:], delta[:], OP.mult)
                outk = ak.tile([128, F], f32, tag="outk", name="outk")
                nc.vector.tensor_tensor(outk[:], hx_t[k][:], mdelta[:], OP.add)
                nc.sync.dma_start(houtT.ap()[bass.ts(k, 128), sl], outk[:])

    nc.compile()
    return nc


def _prep_shared(inputs):
    """Host-side weight prep (shared across cores)."""
    g = lambda k: np.asarray(inputs[k], np.float32)
    Wq1, Wk1, Wv1 = g("Wq1"), g("Wk1"), g("Wv1")
    Wq2, Wk2, Wv2 = g("Wq2"), g("Wk2"), g("Wv2")
    fc_w, fc_b, gate_w, gate_b = g("fc_w"), g("fc_b"), g("gate_w"), g("gate_b")
    gwi, gwh, gbi, gbh = g("gru_wi"), g("gru_wh"), g("gru_bi"), g("gru_bh")

    sh = {}
    sh["wq1"] = np.ascontiguousarray(Wq1.transpose(1, 0, 2).reshape(128, 512))
    sh["wk1"] = np.ascontiguousarray(
        Wk1[1].reshape(2, 128, 64).transpose(1, 0, 2).reshape(128, 128))
    wf = np.einsum("de,kef->kdf", Wv1[1], gwi)           # [8, 256, 384]
    sh["wfu"] = np.ascontiguousarray(
        wf.reshape(8, 2, 128, 384).transpose(2, 1, 0, 3).reshape(128, 6144)).astype(BF)
    sh["wh"] = np.ascontiguousarray(gwh.transpose(1, 0, 2).reshape(128, 3072)).astype(BF)
    sh["wq2"] = np.ascontiguousarray(Wq2.transpose(1, 0, 2).reshape(128, 512)).astype(BF)
    sh["wk2"] = np.ascontiguousarray(Wk2.transpose(1, 0, 2).reshape(128, 512)).astype(BF)
    sh["wv2"] = np.ascontiguousarray(Wv2.transpose(1, 0, 2).reshape(128, 512)).astype(BF)
    fg = np.zeros((64, 256), np.float32)
    fg[:, 0:128] = fc_w
    fg[:, 128:256] = gate_w
    sh["fcg"] = fg.astype(BF)

    brz = np.zeros((128, 16), np.float32)
    bnbh = np.zeros((128, 8), np.float32)
    bnbi = np.zeros((128, 8), np.float32)
    for k in range(8):
        brz[:, 2 * k] = gbi[k, 0:128] + gbh[k, 0:128]
        brz[:, 2 * k + 1] = -(gbi[k, 128:256] + gbh[k, 128:256])
        bnbh[:, k] = gbh[k, 256:384]
        bnbi[:, k] = gbi[k, 256:384]
    sh["b_rz"], sh["b_nbh"], sh["b_nbi"] = brz, bnbh, bnbi
    bfg = np.zeros((128, 2), np.float32)
    bfg[:, 0] = fc_b
    bfg[:, 1] = gate_b
    sh["b_fg"] = bfg
    for k, v in _CONSTS.items():
        sh["c_" + k] = v
    for k in ("c_s1sum", "pq", "r64"):
        sh["f_" + k] = _CONSTS[k].astype(np.float32)
    return sh


def _core_inputs(sh, inp, hx, c):
    s = slice(c * BC, (c + 1) * BC)
    m = dict(sh)
    inpTc = np.ascontiguousarray(inp[s].T)
    m["inpT"] = inpTc.astype(BF)
    m["inpTf"] = inpTc
    hxTc = np.ascontiguousarray(hx[s].T)
    m["hxT"] = hxTc
    m["hxTb"] = hxTc.astype(BF)
    return m


def kernel(**inputs):
    global _PROGRAM
    if _PROGRAM is None:
        _PROGRAM = _build_program()
    nc = _PROGRAM

    inp = np.asarray(inputs["inp"], np.float32)
    hx = np.asarray(inputs["hx"], np.float32)

    sh = _prep_shared(inputs)
    in_maps = [_core_inputs(sh, inp, hx, c) for c in range(NCORES)]

    res = run_bass_kernel_spmd(nc, in_maps, list(range(NCORES)))
    hx_out = np.empty((B, NHID), np.float32)
    mask_full = np.empty((B, NHID), np.float32)
    for c in range(NCORES):
        s = slice(c * BC, (c + 1) * BC)
        hx_out[s] = res.results[c]["houtT"].T
        mask_full[s] = np.repeat(res.results[c]["mask8"].T, 128, axis=1)
    return hx_out, mask_full

